# revision 4
# baseline (speedup 1.0000x reference)
"""Trainium2 Bass kernel for nn_BRCLoss (supervised-contrastive style loss).

Math (per batch sample b, matching the jax reference):
    f = features[b].reshape(24, 4096); fhat = f / ||f||_row
    logits = (fhat @ fhat.T) / 0.1                       # [24, 24]
    exp_logits = exp(logits) * (1 - I)
    log_prob = logits - log(exp_logits.sum(-1))
    mlpp = (mask * log_prob).sum(-1) / (mask.sum(-1) + 1e-6)
    loss = sum_b mean_m(-0.1 * mlpp) / 512               # scalar

`outputs` / `targets` are unused by the reference; only `features`
[512, 2, 12, 4096] f32 matters.  Pure data parallel: 64 samples per core.

Per-core kernel:
  - 13 tiles of [120 rows, 4096] (5 samples each; the last tile re-reads the
    trailing 120 rows, the duplicated sample is zero-weighted).
  - For each tile: PE-transposes 32 f32 chunks [120,128] -> PSUM [128,120],
    downcast-copies to bf16 SBUF (alternating DVE/ACT), then 32 accumulating
    [128,120]x[128,120] bf16 matmuls -> raw per-tile Gram [120,120] (block
    diagonal holds the 5 per-sample 24x24 Grams).
  - Epilogue: row norms come off the Gram diagonal (identity mask + reduce),
    normalization + 1/temperature are applied as a rank-1 outer product of
    rsqrt-norms via K=1 matmuls, then masked exp/log-sum and a weighted
    reduction with host-precomputed masks/weights fold the whole loss
    (including mean/positive-count/validity bookkeeping) into one dot product.
Host sums the 8 per-core scalars.
"""

import os
import sys

import numpy as np

if "/opt/trn_rl_repo" not in sys.path:
    sys.path.insert(0, "/opt/trn_rl_repo")

# Problem constants (hardcoded; kernel.py must be self-contained).
B = 512
NV = 2
NCLS = 12
D = 4096
M = NV * NCLS              # 24 anchor rows per sample
NCORES = 8
SPC = B // NCORES          # 64 samples per core
ROWS = SPC * M             # 1536 feature rows per core
P = 120                    # rows per tile (5 samples)
G5 = P // M                # samples per tile
T = 13                     # tiles per core (12 aligned + 1 overlapping tail)
CH = 128                   # contraction chunk (PE partition limit)
NCH = D // CH              # 32 chunks
QUAD = 4                   # transposed chunks packed per PSUM bank
NQ = NCH // QUAD
TEMP = 0.1
EPS_POS = 1e-6

_ROW_STARTS = [P * t for t in range(T - 1)] + [ROWS - P]

_compiled = None           # (nc, const_in_map)
LAST_RESULTS = None        # BassKernelResults of the most recent run


def _host_consts():
    """Masks/weights shared by every core (the per-core sample grid is identical)."""
    i = np.arange(NCLS)
    graph = (np.abs(i[:, None] - i[None, :]) <= 1).astype(np.float32)   # [12,12]
    eye24 = np.eye(M, dtype=np.float32)
    mask24 = np.tile(graph, (NV, NV)) * (1.0 - eye24)                    # positives
    blk = np.kron(np.eye(G5, dtype=np.float32), np.ones((M, M), np.float32))
    m0 = (blk * (1.0 - np.eye(P, dtype=np.float32))).astype(np.float32)  # denom mask
    pm = np.kron(np.eye(G5, dtype=np.float32), mask24).astype(np.float32)
    im = np.eye(P, dtype=np.float32)
    msum = np.tile(mask24.sum(1), G5).astype(np.float64)                 # [120], 3 or 5
    alpha = -TEMP / ((msum + EPS_POS) * M * B)                           # per-row weight
    valid = np.ones((P, T), np.float64)
    valid[:M, T - 1] = 0.0   # last tile re-reads sample 59 -> zero weight
    w1 = (alpha[:, None] * valid).astype(np.float32)
    w2 = ((-alpha * msum)[:, None] * valid).astype(np.float32)
    return {"m0": m0, "pm": pm, "im": im, "w1": w1, "w2": w2}


def _build():
    from contextlib import ExitStack

    from concourse import bacc, bass, masks, mybir, tile

    f32 = mybir.dt.float32
    bf16 = mybir.dt.bfloat16
    AX = mybir.AxisListType
    ALU = mybir.AluOpType
    ACTF = mybir.ActivationFunctionType

    nc = bacc.Bacc("TRN2", target_bir_lowering=False, debug=False,
                   num_devices=NCORES)

    f_dram = nc.dram_tensor("f", (ROWS, D), f32, kind="ExternalInput")
    m0_dram = nc.dram_tensor("m0", (P, P), f32, kind="ExternalInput")
    pm_dram = nc.dram_tensor("pm", (P, P), f32, kind="ExternalInput")
    im_dram = nc.dram_tensor("im", (P, P), f32, kind="ExternalInput")
    w1_dram = nc.dram_tensor("w1", (P, T), f32, kind="ExternalInput")
    w2_dram = nc.dram_tensor("w2", (P, T), f32, kind="ExternalInput")
    out_dram = nc.dram_tensor("out", (1, 1), f32, kind="ExternalOutput")

    with ExitStack() as ctx:
        tc = ctx.enter_context(tile.TileContext(nc))
        consts = ctx.enter_context(tc.tile_pool(name="consts", bufs=1))
        fpool = ctx.enter_context(tc.tile_pool(name="fpool", bufs=3))
        tcpool = ctx.enter_context(tc.tile_pool(name="tcpool", bufs=4))
        epool = ctx.enter_context(tc.tile_pool(name="epool", bufs=1))
        work = ctx.enter_context(tc.tile_pool(name="work", bufs=1))
        tpsum = ctx.enter_context(
            tc.tile_pool(name="tpsum", bufs=2, space=bass.MemorySpace.PSUM))
        gpsum = ctx.enter_context(
            tc.tile_pool(name="gpsum", bufs=2, space=bass.MemorySpace.PSUM))
        rpsum = ctx.enter_context(
            tc.tile_pool(name="rpsum", bufs=1, space=bass.MemorySpace.PSUM))

        ident = consts.tile([128, 128], f32, tag="ident")
        masks.make_identity(nc, ident[:])
        m0_t = consts.tile([P, P], f32, tag="m0")
        pm_t = consts.tile([P, P], f32, tag="pm")
        im_t = consts.tile([P, P], f32, tag="im")
        w1_t = consts.tile([P, T], f32, tag="w1")
        w2_t = consts.tile([P, T], f32, tag="w2")
        nc.sync.dma_start(m0_t[:], m0_dram[:, :])
        nc.sync.dma_start(pm_t[:], pm_dram[:, :])
        nc.sync.dma_start(im_t[:], im_dram[:, :])
        nc.sync.dma_start(w1_t[:], w1_dram[:, :])
        nc.sync.dma_start(w2_t[:], w2_dram[:, :])

        e_t = epool.tile([P, T * P], f32, tag="e")   # raw Gram blocks, f32

        for t in range(T):
            r0 = _ROW_STARTS[t]
            ft = fpool.tile([P, D], f32, tag="f")
            nc.sync.dma_start(ft[:], f_dram[r0:r0 + P, :])
            g = gpsum.tile([P, P], f32, tag="g")
            for q in range(NQ):
                tp = tpsum.tile([128, QUAD * P], f32, tag="tp")
                for j in range(QUAD):
                    c = q * QUAD + j
                    nc.tensor.transpose(
                        tp[:, j * P:(j + 1) * P],
                        ft[:, c * CH:(c + 1) * CH],
                        ident[:P, :P],
                    )
                tcs = tcpool.tile([128, QUAD * P], bf16, tag="tc")
                if q % 2 == 0:
                    nc.vector.tensor_copy(tcs[:], tp[:])
                else:
                    nc.scalar.copy(tcs[:], tp[:])
                for j in range(QUAD):
                    c = q * QUAD + j
                    sl = tcs[:, j * P:(j + 1) * P]
                    nc.tensor.matmul(g[:], sl, sl,
                                     start=(c == 0), stop=(c == NCH - 1))
            nc.vector.tensor_copy(e_t[:, t * P:(t + 1) * P], g[:])

        # ---- epilogue ----
        e3 = e_t[:].rearrange("p (t c) -> p t c", t=T)
        im_bc = im_t[:, None, :].broadcast_to([P, T, P])
        m0_bc = m0_t[:, None, :].broadcast_to([P, T, P])
        pm_bc = pm_t[:, None, :].broadcast_to([P, T, P])

        tmp = work.tile([P, T * P], f32, tag="tmp")
        tmp3 = tmp[:].rearrange("p (t c) -> p t c", t=T)
        nc.vector.tensor_tensor(tmp3, e3, im_bc, ALU.mult)
        d2 = work.tile([P, T], f32, tag="d2")        # squared row norms
        nc.vector.tensor_reduce(d2[:], tmp3, axis=AX.X, op=ALU.add)
        sroot = work.tile([P, T], f32, tag="sroot")
        nc.scalar.activation(sroot[:], d2[:], ACTF.Sqrt, scale=TEMP)
        rnx = work.tile([P, T], f32, tag="rnx")      # sqrt(10)/||f_row||
        nc.vector.reciprocal(rnx[:], sroot[:])

        rnT_ps = gpsum.tile([T, P], f32, tag="g")
        nc.tensor.transpose(rnT_ps[:], rnx[:], ident[:P, :P])
        rnT = work.tile([T, P], f32, tag="rnT")
        nc.vector.tensor_copy(rnT[:], rnT_ps[:])

        selpool = ctx.enter_context(tc.tile_pool(name="selpool", bufs=2))
        rnc_ps = rpsum.tile([P, T, 128], f32, tag="rnc")
        for t in range(T):
            # sel zeroes every row of rnT except row t, so the K=13 matmul
            # reduces to the rank-1 outer product rnx[:,t] (x) rnx[:,t].
            sel = selpool.tile([T, P], f32, tag="sel")
            nc.vector.tensor_scalar_mul(sel[:], rnT[:], im_t[:T, t:t + 1])
            nc.tensor.matmul(rnc_ps[:, t, :P], sel[:], rnT[:],
                             start=True, stop=True)

        logit = work.tile([P, T * P], f32, tag="logit")
        l3 = logit[:].rearrange("p (t c) -> p t c", t=T)
        nc.vector.tensor_tensor(l3, e3, rnc_ps[:, :, :P], ALU.mult)

        x_t = work.tile([P, T * P], f32, tag="x")
        nc.scalar.activation(x_t[:], logit[:], ACTF.Exp)
        x3 = x_t[:].rearrange("p (t c) -> p t c", t=T)
        xm = work.tile([P, T * P], f32, tag="xm")
        xm3 = xm[:].rearrange("p (t c) -> p t c", t=T)
        nc.vector.tensor_tensor(xm3, x3, m0_bc, ALU.mult)
        s_t = work.tile([P, T], f32, tag="s")
        nc.vector.tensor_reduce(s_t[:], xm3, axis=AX.X, op=ALU.add)
        ld = work.tile([P, T], f32, tag="ld")
        nc.scalar.activation(ld[:], s_t[:], ACTF.Ln)

        lp = work.tile([P, T * P], f32, tag="lp")
        lp3 = lp[:].rearrange("p (t c) -> p t c", t=T)
        nc.vector.tensor_tensor(lp3, l3, pm_bc, ALU.mult)
        t1 = work.tile([P, T], f32, tag="t1")
        nc.vector.tensor_reduce(t1[:], lp3, axis=AX.X, op=ALU.add)

        z1 = work.tile([P, T], f32, tag="z1")
        nc.vector.tensor_tensor(z1[:], t1[:], w1_t[:], ALU.mult)
        z2 = work.tile([P, T], f32, tag="z2")
        nc.vector.tensor_tensor(z2[:], ld[:], w2_t[:], ALU.mult)
        zs = work.tile([P, T], f32, tag="zs")
        nc.vector.tensor_add(zs[:], z1[:], z2[:])
        zc = work.tile([P, 1], f32, tag="zc")
        nc.vector.tensor_reduce(zc[:], zs[:], axis=AX.X, op=ALU.add)

        ones = work.tile([P, 1], f32, tag="ones")
        nc.vector.memset(ones[:], 1.0)
        tot_ps = gpsum.tile([1, 1], f32, tag="g")
        nc.tensor.matmul(tot_ps[:, :], zc[:], ones[:], start=True, stop=True)
        tot = work.tile([1, 1], f32, tag="tot")
        nc.vector.tensor_copy(tot[:], tot_ps[:, :])
        nc.sync.dma_start(out_dram[:, :], tot[:])

    nc.compile()
    return nc


def _ensure_axon_hooks():
    """Provide antenv.axon_hooks if the image lacks it (NTFF profiling shim).

    Mirrors trn_agent_boot.trn_boot: the hook drives NRT profiling via the
    libaxon_pjrt.so C ABI.  If anything is missing we register a None hook,
    which makes bass_utils skip tracing gracefully instead of crashing.
    """
    try:
        import antenv.axon_hooks  # noqa: F401
        return
    except ImportError:
        pass
    import contextlib
    import ctypes
    import types

    import antenv

    hook = None
    so_path = "/opt/axon/libaxon_pjrt.so"
    try:
        lib = ctypes.CDLL(so_path)
        if hasattr(lib, "axon_start_nrt_profile"):
            lib.axon_start_nrt_profile.argtypes = [
                ctypes.POINTER(ctypes.c_int64), ctypes.c_size_t]
            lib.axon_start_nrt_profile.restype = ctypes.c_int64
            lib.axon_stop_nrt_profile.argtypes = [ctypes.c_char_p]
            lib.axon_stop_nrt_profile.restype = ctypes.c_int64

            @contextlib.contextmanager
            def _hook(output_dir, device_ids):
                import jax
                jax.devices()
                if device_ids:
                    ids = (ctypes.c_int64 * len(device_ids))(*device_ids)
                    rc = lib.axon_start_nrt_profile(ids, len(device_ids))
                else:
                    rc = lib.axon_start_nrt_profile(None, 0)
                if rc != 0:
                    raise RuntimeError(f"axon_start_nrt_profile rc={rc}")
                try:
                    yield
                finally:
                    n = lib.axon_stop_nrt_profile(str(output_dir).encode())
                    print(f"profile: {n} file(s) written to {output_dir}",
                          file=sys.stderr)

            hook = _hook
    except OSError:
        pass

    mod = types.ModuleType("antenv.axon_hooks")
    state = {"hook": hook}
    mod.get_axon_ntff_profile_hook = lambda: state["hook"]
    mod.set_axon_ntff_profile_hook = lambda h: state.__setitem__("hook", h)
    sys.modules["antenv.axon_hooks"] = mod
    antenv.axon_hooks = mod


def kernel(**inputs):
    global _compiled, LAST_RESULTS
    from concourse import bass_utils

    feats = np.ascontiguousarray(
        np.asarray(inputs["features"], dtype=np.float32).reshape(B * M, D))

    if _compiled is None:
        _compiled = (_build(), _host_consts())
    nc, consts = _compiled

    in_maps = []
    for k in range(NCORES):
        im = dict(consts)
        im["f"] = feats[k * ROWS:(k + 1) * ROWS]
        in_maps.append(im)

    trace = bool(os.environ.get("BASS_TRACE"))
    if trace:
        _ensure_axon_hooks()
    try:
        res = bass_utils.run_bass_kernel_spmd(
            nc, in_maps, core_ids=list(range(NCORES)), trace=trace)
    except Exception:
        if not trace:
            raise
        # Tracing plumbing failed; rerun untraced so the result is still valid.
        os.environ["BASS_NEVER_TRACE"] = "1"
        try:
            res = bass_utils.run_bass_kernel_spmd(
                nc, in_maps, core_ids=list(range(NCORES)), trace=False)
        finally:
            del os.environ["BASS_NEVER_TRACE"]
    LAST_RESULTS = res
    total = float(np.sum([np.float64(r["out"][0, 0]) for r in res.results]))
    return np.array(total, dtype=np.float32)


# revision 11
# speedup vs baseline: 1.0939x; 1.0939x over previous
"""Trainium2 Bass kernel for nn_BRCLoss (supervised-contrastive style loss).

Math (per batch sample b, matching the jax reference):
    f = features[b].reshape(24, 4096); fhat = f / ||f||_row
    logits = (fhat @ fhat.T) / 0.1                       # [24, 24]
    exp_logits = exp(logits) * (1 - I)
    log_prob = logits - log(exp_logits.sum(-1))
    mlpp = (mask * log_prob).sum(-1) / (mask.sum(-1) + 1e-6)
    loss = sum_b mean_m(-0.1 * mlpp) / 512               # scalar

`outputs` / `targets` are unused by the reference; only `features`
[512, 2, 12, 4096] f32 matters.  Pure data parallel: 64 samples per core.

Per-core kernel:
  - 13 tiles of [120 rows, 4096] (5 samples each; the last tile re-reads the
    trailing 120 rows, the duplicated sample is zero-weighted).
  - For each tile: PE-transposes 32 f32 chunks [120,128] -> PSUM [128,120],
    downcast-copies to bf16 SBUF (alternating DVE/ACT), then 32 accumulating
    [128,120]x[128,120] bf16 matmuls -> raw per-tile Gram [120,120] (block
    diagonal holds the 5 per-sample 24x24 Grams).
  - Epilogue: row norms come off the Gram diagonal (identity mask + reduce),
    normalization + 1/temperature are applied as a rank-1 outer product of
    rsqrt-norms via K=1 matmuls, then masked exp/log-sum and a weighted
    reduction with host-precomputed masks/weights fold the whole loss
    (including mean/positive-count/validity bookkeeping) into one dot product.
Host sums the 8 per-core scalars.
"""

import os
import sys

import numpy as np

if "/opt/trn_rl_repo" not in sys.path:
    sys.path.insert(0, "/opt/trn_rl_repo")

# Problem constants (hardcoded; kernel.py must be self-contained).
B = 512
NV = 2
NCLS = 12
D = 4096
M = NV * NCLS              # 24 anchor rows per sample
NCORES = 8
SPC = B // NCORES          # 64 samples per core
ROWS = SPC * M             # 1536 feature rows per core
P = 120                    # rows per tile (5 samples)
G5 = P // M                # samples per tile
T = 13                     # tiles per core (12 aligned + 1 overlapping tail)
CH = 128                   # contraction chunk (PE partition limit)
NCH = D // CH              # 32 chunks
QUAD = 4                   # transposed chunks packed per PSUM bank
NQ = NCH // QUAD
TEMP = 0.1
EPS_POS = 1e-6

_ROW_STARTS = [P * t for t in range(T - 1)] + [ROWS - P]

_compiled = None           # (nc, const_in_map)
LAST_RESULTS = None        # BassKernelResults of the most recent run


def _host_consts():
    """Masks/weights shared by every core (the per-core sample grid is identical)."""
    i = np.arange(NCLS)
    graph = (np.abs(i[:, None] - i[None, :]) <= 1).astype(np.float32)   # [12,12]
    eye24 = np.eye(M, dtype=np.float32)
    mask24 = np.tile(graph, (NV, NV)) * (1.0 - eye24)                    # positives
    blk = np.kron(np.eye(G5, dtype=np.float32), np.ones((M, M), np.float32))
    m0 = (blk * (1.0 - np.eye(P, dtype=np.float32))).astype(np.float32)  # denom mask
    pm = np.kron(np.eye(G5, dtype=np.float32), mask24).astype(np.float32)
    im = np.eye(P, dtype=np.float32)
    msum = np.tile(mask24.sum(1), G5).astype(np.float64)                 # [120], 3 or 5
    alpha = -TEMP / ((msum + EPS_POS) * M * B)                           # per-row weight
    valid = np.ones((P, T), np.float64)
    valid[:M, T - 1] = 0.0   # last tile re-reads sample 59 -> zero weight
    w1 = (alpha[:, None] * valid).astype(np.float32)
    w2 = ((-alpha * msum)[:, None] * valid).astype(np.float32)
    return {"m0": m0, "pm": pm, "im": im, "w1": w1, "w2": w2}


def _build():
    from contextlib import ExitStack

    from concourse import bacc, bass, masks, mybir, tile

    f32 = mybir.dt.float32
    bf16 = mybir.dt.bfloat16
    AX = mybir.AxisListType
    ALU = mybir.AluOpType
    ACTF = mybir.ActivationFunctionType

    nc = bacc.Bacc("TRN2", target_bir_lowering=False, debug=False,
                   num_devices=NCORES)

    f_dram = nc.dram_tensor("f", (ROWS, D), f32, kind="ExternalInput")
    m0_dram = nc.dram_tensor("m0", (P, P), f32, kind="ExternalInput")
    pm_dram = nc.dram_tensor("pm", (P, P), f32, kind="ExternalInput")
    im_dram = nc.dram_tensor("im", (P, P), f32, kind="ExternalInput")
    w1_dram = nc.dram_tensor("w1", (P, T), f32, kind="ExternalInput")
    w2_dram = nc.dram_tensor("w2", (P, T), f32, kind="ExternalInput")
    out_dram = nc.dram_tensor("out", (1, 1), f32, kind="ExternalOutput")

    DSPLIT = 4                 # DMAs per feature tile
    DCOLS = D // DSPLIT

    with ExitStack() as ctx:
        tc = ctx.enter_context(tile.TileContext(nc))
        consts = ctx.enter_context(tc.tile_pool(name="consts", bufs=1))
        fpool = ctx.enter_context(tc.tile_pool(name="fpool", bufs=3))
        tcpool = ctx.enter_context(tc.tile_pool(name="tcpool", bufs=6))
        work = ctx.enter_context(tc.tile_pool(name="work", bufs=1))
        lwork = ctx.enter_context(tc.tile_pool(name="lwork", bufs=2))
        small = ctx.enter_context(tc.tile_pool(name="small", bufs=2))
        tpsum = ctx.enter_context(
            tc.tile_pool(name="tpsum", bufs=3, space=bass.MemorySpace.PSUM))
        gpsum = ctx.enter_context(
            tc.tile_pool(name="gpsum", bufs=2, space=bass.MemorySpace.PSUM))
        rpsum = ctx.enter_context(
            tc.tile_pool(name="rpsum", bufs=2, space=bass.MemorySpace.PSUM))

        ident = consts.tile([128, 128], f32, tag="ident")
        masks.make_identity(nc, ident[:])
        m0_t = consts.tile([P, P], f32, tag="m0")
        pm_t = consts.tile([P, P], f32, tag="pm")
        im_t = consts.tile([P, P], f32, tag="im")
        w1_t = consts.tile([P, T], f32, tag="w1")
        w2_t = consts.tile([P, T], f32, tag="w2")
        nc.sync.dma_start(m0_t[:], m0_dram[:, :])
        nc.sync.dma_start(pm_t[:], pm_dram[:, :])
        nc.sync.dma_start(im_t[:], im_dram[:, :])
        nc.sync.dma_start(w1_t[:], w1_dram[:, :])
        nc.sync.dma_start(w2_t[:], w2_dram[:, :])

        # Preload the exp/ln activation table set while DMA streams.
        warm = consts.tile([1, 2], f32, tag="warm")
        nc.vector.memset(warm[:], 1.0)
        nc.scalar.activation(warm[:, 1:2], warm[:, 0:1], ACTF.Exp)

        t1cols = work.tile([P, T], f32, tag="t1cols")   # sum(mask*logits) per tile
        ldcols = work.tile([P, T], f32, tag="ldcols")   # log-denominators per tile

        for t in range(T):
            r0 = _ROW_STARTS[t]
            ft = fpool.tile([P, D], f32, tag="f")
            nc.sync.dma_start(ft[:], f_dram[r0:r0 + P, :])
            g = gpsum.tile([P, P], f32, tag="g")
            for q in range(NQ):
                tp = tpsum.tile([128, QUAD * P], f32, tag="tp")
                for j in range(QUAD):
                    c = q * QUAD + j
                    nc.tensor.transpose(
                        tp[:, j * P:(j + 1) * P],
                        ft[:, c * CH:(c + 1) * CH],
                        ident[:P, :P],
                    )
                tcs = tcpool.tile([128, QUAD * P], bf16, tag="tc")
                if q % 2 == 0:
                    nc.vector.tensor_copy(tcs[:], tp[:])
                else:
                    nc.scalar.copy(tcs[:], tp[:])
                for j in range(QUAD):
                    c = q * QUAD + j
                    sl = tcs[:, j * P:(j + 1) * P]
                    nc.tensor.matmul(g[:], sl, sl,
                                     start=(c == 0), stop=(c == NCH - 1))

            # Per-tile epilogue.  d2 = diag(G); rnx = (0.1*d2)^-0.5 via exp/ln
            # (Sqrt lives in a different ACT table set -> avoid switches).
            # logits L = diag(rnx) @ G @ diag(rnx): the column scaling runs on
            # the PE as G @ diag(rnx) (G is symmetric so lhsT=G works), the row
            # scaling as a per-partition tensor_scalar.
            eg = lwork.tile([P, P], f32, tag="eg")
            nc.vector.tensor_copy(eg[:], g[:])
            scr = lwork.tile([P, P], f32, tag="scr")
            nc.vector.tensor_tensor(scr[:], eg[:], im_t[:], ALU.mult)
            d2 = small.tile([P, 1], f32, tag="d2")
            nc.vector.tensor_reduce(d2[:], scr[:], axis=AX.X, op=ALU.add)
            lnv = small.tile([P, 1], f32, tag="lnv")
            nc.scalar.activation(lnv[:], d2[:], ACTF.Ln, scale=TEMP)
            rnx = small.tile([P, 1], f32, tag="rnx")
            nc.scalar.activation(rnx[:], lnv[:], ACTF.Exp, scale=-0.5)
            drn = lwork.tile([P, P], f32, tag="drn")
            nc.vector.tensor_scalar_mul(drn[:], im_t[:], rnx[:])
            h_ps = rpsum.tile([P, P], f32, tag="r")
            nc.tensor.matmul(h_ps[:], eg[:], drn[:], start=True, stop=True)
            lt = lwork.tile([P, P], f32, tag="lt")
            nc.vector.tensor_scalar_mul(lt[:], h_ps[:], rnx[:])

            xt = lwork.tile([P, P], f32, tag="xt")
            nc.scalar.activation(xt[:], lt[:], ACTF.Exp)
            xm = lwork.tile([P, P], f32, tag="xm")
            nc.vector.tensor_tensor(xm[:], xt[:], m0_t[:], ALU.mult)
            st = small.tile([P, 1], f32, tag="st")
            nc.vector.tensor_reduce(st[:], xm[:], axis=AX.X, op=ALU.add)
            nc.scalar.activation(ldcols[:, t:t + 1], st[:], ACTF.Ln)
            lp = lwork.tile([P, P], f32, tag="lp")
            nc.vector.tensor_tensor(lp[:], lt[:], pm_t[:], ALU.mult)
            nc.vector.tensor_reduce(t1cols[:, t:t + 1], lp[:], axis=AX.X,
                                    op=ALU.add)

        # ---- final weighted reduction ----
        z1 = work.tile([P, T], f32, tag="z1")
        nc.vector.tensor_tensor(z1[:], t1cols[:], w1_t[:], ALU.mult)
        z2 = work.tile([P, T], f32, tag="z2")
        nc.vector.tensor_tensor(z2[:], ldcols[:], w2_t[:], ALU.mult)
        zs = work.tile([P, T], f32, tag="zs")
        nc.vector.tensor_add(zs[:], z1[:], z2[:])
        zc = work.tile([P, 1], f32, tag="zc")
        nc.vector.tensor_reduce(zc[:], zs[:], axis=AX.X, op=ALU.add)

        ones = work.tile([P, 1], f32, tag="ones")
        nc.vector.memset(ones[:], 1.0)
        tot_ps = gpsum.tile([1, 1], f32, tag="g")
        nc.tensor.matmul(tot_ps[:, :], zc[:], ones[:], start=True, stop=True)
        tot = work.tile([1, 1], f32, tag="tot")
        nc.vector.tensor_copy(tot[:], tot_ps[:, :])
        nc.sync.dma_start(out_dram[:, :], tot[:])

    nc.compile()
    return nc


def _ensure_axon_hooks():
    """Provide antenv.axon_hooks if the image lacks it (NTFF profiling shim).

    Mirrors trn_agent_boot.trn_boot: the hook drives NRT profiling via the
    libaxon_pjrt.so C ABI.  If anything is missing we register a None hook,
    which makes bass_utils skip tracing gracefully instead of crashing.
    """
    try:
        import antenv.axon_hooks  # noqa: F401
        return
    except ImportError:
        pass
    import contextlib
    import ctypes
    import types

    import antenv

    hook = None
    so_path = "/opt/axon/libaxon_pjrt.so"
    try:
        lib = ctypes.CDLL(so_path)
        if hasattr(lib, "axon_start_nrt_profile"):
            lib.axon_start_nrt_profile.argtypes = [
                ctypes.POINTER(ctypes.c_int64), ctypes.c_size_t]
            lib.axon_start_nrt_profile.restype = ctypes.c_int64
            lib.axon_stop_nrt_profile.argtypes = [ctypes.c_char_p]
            lib.axon_stop_nrt_profile.restype = ctypes.c_int64

            @contextlib.contextmanager
            def _hook(output_dir, device_ids):
                import jax
                jax.devices()
                if device_ids:
                    ids = (ctypes.c_int64 * len(device_ids))(*device_ids)
                    rc = lib.axon_start_nrt_profile(ids, len(device_ids))
                else:
                    rc = lib.axon_start_nrt_profile(None, 0)
                if rc != 0:
                    raise RuntimeError(f"axon_start_nrt_profile rc={rc}")
                try:
                    yield
                finally:
                    n = lib.axon_stop_nrt_profile(str(output_dir).encode())
                    print(f"profile: {n} file(s) written to {output_dir}",
                          file=sys.stderr)

            hook = _hook
    except OSError:
        pass

    mod = types.ModuleType("antenv.axon_hooks")
    state = {"hook": hook}
    mod.get_axon_ntff_profile_hook = lambda: state["hook"]
    mod.set_axon_ntff_profile_hook = lambda h: state.__setitem__("hook", h)
    sys.modules["antenv.axon_hooks"] = mod
    antenv.axon_hooks = mod


def kernel(**inputs):
    global _compiled, LAST_RESULTS
    from concourse import bass_utils

    feats = np.ascontiguousarray(
        np.asarray(inputs["features"], dtype=np.float32).reshape(B * M, D))

    if _compiled is None:
        _compiled = (_build(), _host_consts())
    nc, consts = _compiled

    in_maps = []
    for k in range(NCORES):
        im = dict(consts)
        im["f"] = feats[k * ROWS:(k + 1) * ROWS]
        in_maps.append(im)

    trace = bool(os.environ.get("BASS_TRACE"))
    if trace:
        _ensure_axon_hooks()
    try:
        res = bass_utils.run_bass_kernel_spmd(
            nc, in_maps, core_ids=list(range(NCORES)), trace=trace)
    except Exception:
        if not trace:
            raise
        # Tracing plumbing failed; rerun untraced so the result is still valid.
        os.environ["BASS_NEVER_TRACE"] = "1"
        try:
            res = bass_utils.run_bass_kernel_spmd(
                nc, in_maps, core_ids=list(range(NCORES)), trace=False)
        finally:
            del os.environ["BASS_NEVER_TRACE"]
    LAST_RESULTS = res
    total = float(np.sum([np.float64(r["out"][0, 0]) for r in res.results]))
    return np.array(total, dtype=np.float32)


# revision 13
# speedup vs baseline: 1.1855x; 1.0837x over previous
"""Trainium2 Bass kernel for nn_BRCLoss (supervised-contrastive style loss).

Math (per batch sample b, matching the jax reference):
    f = features[b].reshape(24, 4096); fhat = f / ||f||_row
    logits = (fhat @ fhat.T) / 0.1                       # [24, 24]
    exp_logits = exp(logits) * (1 - I)
    log_prob = logits - log(exp_logits.sum(-1))
    mlpp = (mask * log_prob).sum(-1) / (mask.sum(-1) + 1e-6)
    loss = sum_b mean_m(-0.1 * mlpp) / 512               # scalar

`outputs` / `targets` are unused by the reference; only `features`
[512, 2, 12, 4096] f32 matters.  Pure data parallel: 64 samples per core.

Per-core kernel:
  - 13 tiles of [120 rows, 4096] (5 samples each; the last tile re-reads the
    trailing 120 rows, the duplicated sample is zero-weighted).
  - For each tile: PE-transposes 32 f32 chunks [120,128] -> PSUM [128,120],
    downcast-copies to bf16 SBUF (alternating DVE/ACT), then 32 accumulating
    [128,120]x[128,120] bf16 matmuls -> raw per-tile Gram [120,120] (block
    diagonal holds the 5 per-sample 24x24 Grams).
  - Epilogue: row norms come off the Gram diagonal (identity mask + reduce),
    normalization + 1/temperature are applied as a rank-1 outer product of
    rsqrt-norms via K=1 matmuls, then masked exp/log-sum and a weighted
    reduction with host-precomputed masks/weights fold the whole loss
    (including mean/positive-count/validity bookkeeping) into one dot product.
Host sums the 8 per-core scalars.
"""

import os
import sys

import numpy as np

if "/opt/trn_rl_repo" not in sys.path:
    sys.path.insert(0, "/opt/trn_rl_repo")

# Problem constants (hardcoded; kernel.py must be self-contained).
B = 512
NV = 2
NCLS = 12
D = 4096
M = NV * NCLS              # 24 anchor rows per sample
NCORES = 8
SPC = B // NCORES          # 64 samples per core
ROWS = SPC * M             # 1536 feature rows per core
P = 120                    # rows per tile (5 samples)
G5 = P // M                # samples per tile
T = 13                     # tiles per core (12 aligned + 1 overlapping tail)
CH = 128                   # contraction chunk (PE partition limit)
NCH = D // CH              # 32 chunks
QUAD = 4                   # transposed chunks packed per PSUM bank
NQ = NCH // QUAD
TEMP = 0.1
EPS_POS = 1e-6

_ROW_STARTS = [P * t for t in range(T - 1)] + [ROWS - P]

_compiled = None           # (nc, const_in_map)
LAST_RESULTS = None        # BassKernelResults of the most recent run


def _host_consts():
    """Masks/weights shared by every core (the per-core sample grid is identical)."""
    i = np.arange(NCLS)
    graph = (np.abs(i[:, None] - i[None, :]) <= 1).astype(np.float32)   # [12,12]
    eye24 = np.eye(M, dtype=np.float32)
    mask24 = np.tile(graph, (NV, NV)) * (1.0 - eye24)                    # positives
    blk = np.kron(np.eye(G5, dtype=np.float32), np.ones((M, M), np.float32))
    m0 = (blk * (1.0 - np.eye(P, dtype=np.float32))).astype(np.float32)  # denom mask
    pm = np.kron(np.eye(G5, dtype=np.float32), mask24).astype(np.float32)
    im = np.eye(P, dtype=np.float32)
    msum = np.tile(mask24.sum(1), G5).astype(np.float64)                 # [120], 3 or 5
    alpha = -TEMP / ((msum + EPS_POS) * M * B)                           # per-row weight
    valid = np.ones((P, T), np.float64)
    valid[:M, T - 1] = 0.0   # last tile re-reads sample 59 -> zero weight
    w1 = (alpha[:, None] * valid).astype(np.float32)
    w2 = ((-alpha * msum)[:, None] * valid).astype(np.float32)
    return {"m0": m0, "pm": pm, "im": im, "w1": w1, "w2": w2}


def _build():
    from contextlib import ExitStack

    from concourse import bacc, bass, masks, mybir, tile

    f32 = mybir.dt.float32
    bf16 = mybir.dt.bfloat16
    AX = mybir.AxisListType
    ALU = mybir.AluOpType
    ACTF = mybir.ActivationFunctionType

    nc = bacc.Bacc("TRN2", target_bir_lowering=False, debug=False,
                   num_devices=NCORES)

    f_dram = nc.dram_tensor("f", (ROWS, D), f32, kind="ExternalInput")
    m0_dram = nc.dram_tensor("m0", (P, P), f32, kind="ExternalInput")
    pm_dram = nc.dram_tensor("pm", (P, P), f32, kind="ExternalInput")
    im_dram = nc.dram_tensor("im", (P, P), f32, kind="ExternalInput")
    w1_dram = nc.dram_tensor("w1", (P, T), f32, kind="ExternalInput")
    w2_dram = nc.dram_tensor("w2", (P, T), f32, kind="ExternalInput")
    out_dram = nc.dram_tensor("out", (1, 1), f32, kind="ExternalOutput")

    DSPLIT = 4                 # DMAs per feature tile
    DCOLS = D // DSPLIT

    with ExitStack() as ctx:
        tc = ctx.enter_context(tile.TileContext(nc))
        consts = ctx.enter_context(tc.tile_pool(name="consts", bufs=1))
        fpool = ctx.enter_context(tc.tile_pool(name="fpool", bufs=3))
        tcpool = ctx.enter_context(tc.tile_pool(name="tcpool", bufs=6))
        work = ctx.enter_context(tc.tile_pool(name="work", bufs=1))
        lwork = ctx.enter_context(tc.tile_pool(name="lwork", bufs=2))
        small = ctx.enter_context(tc.tile_pool(name="small", bufs=2))
        tpsum = ctx.enter_context(
            tc.tile_pool(name="tpsum", bufs=3, space=bass.MemorySpace.PSUM))
        gpsum = ctx.enter_context(
            tc.tile_pool(name="gpsum", bufs=2, space=bass.MemorySpace.PSUM))
        rpsum = ctx.enter_context(
            tc.tile_pool(name="rpsum", bufs=2, space=bass.MemorySpace.PSUM))

        # Feature loads lead the program so the sync-ring FIFO starts streaming
        # them immediately; constants ride the scalar HWDGE ring in parallel.
        ftiles = []
        for t in range(T):
            r0 = _ROW_STARTS[t]
            ft = fpool.tile([P, D], f32, tag="f")
            if t < 3:
                for q in range(DSPLIT):
                    nc.sync.dma_start(ft[:, q * DCOLS:(q + 1) * DCOLS],
                                      f_dram[r0:r0 + P, q * DCOLS:(q + 1) * DCOLS])
            ftiles.append(ft)

        ident = consts.tile([128, 128], f32, tag="ident")
        masks.make_identity(nc, ident[:])
        m0_t = consts.tile([P, P], f32, tag="m0")
        pm_t = consts.tile([P, P], f32, tag="pm")
        im_t = consts.tile([P, P], f32, tag="im")
        w1_t = consts.tile([P, T], f32, tag="w1")
        w2_t = consts.tile([P, T], f32, tag="w2")
        nc.scalar.dma_start(m0_t[:], m0_dram[:, :])
        nc.scalar.dma_start(pm_t[:], pm_dram[:, :])
        nc.scalar.dma_start(im_t[:], im_dram[:, :])
        nc.scalar.dma_start(w1_t[:], w1_dram[:, :])
        nc.scalar.dma_start(w2_t[:], w2_dram[:, :])

        # Preload the exp/ln activation table set while DMA streams.
        warm = consts.tile([1, 2], f32, tag="warm")
        nc.vector.memset(warm[:], 1.0)
        nc.scalar.activation(warm[:, 1:2], warm[:, 0:1], ACTF.Exp)

        t1cols = work.tile([P, T], f32, tag="t1cols")   # sum(mask*logits) per tile
        ldcols = work.tile([P, T], f32, tag="ldcols")   # log-denominators per tile

        for t in range(T):
            ft = ftiles[t]
            if t >= 3:
                r0 = _ROW_STARTS[t]
                for q in range(DSPLIT):
                    nc.sync.dma_start(ft[:, q * DCOLS:(q + 1) * DCOLS],
                                      f_dram[r0:r0 + P, q * DCOLS:(q + 1) * DCOLS])
            g = gpsum.tile([P, P], f32, tag="g")
            for q in range(NQ):
                tp = tpsum.tile([128, QUAD * P], f32, tag="tp")
                for j in range(QUAD):
                    c = q * QUAD + j
                    nc.tensor.transpose(
                        tp[:, j * P:(j + 1) * P],
                        ft[:, c * CH:(c + 1) * CH],
                        ident[:P, :P],
                    )
                tcs = tcpool.tile([128, QUAD * P], bf16, tag="tc")
                if q % 2 == 0:
                    nc.vector.tensor_copy(tcs[:], tp[:])
                else:
                    nc.scalar.copy(tcs[:], tp[:])
                for j in range(QUAD):
                    c = q * QUAD + j
                    sl = tcs[:, j * P:(j + 1) * P]
                    nc.tensor.matmul(g[:], sl, sl,
                                     start=(c == 0), stop=(c == NCH - 1))

            # Per-tile epilogue.  d2 = diag(G); rnx = (0.1*d2)^-0.5 via exp/ln
            # (Sqrt lives in a different ACT table set -> avoid switches).
            # logits L = diag(rnx) @ G @ diag(rnx): the column scaling runs on
            # the PE as G @ diag(rnx) (G is symmetric so lhsT=G works), the row
            # scaling as a per-partition tensor_scalar.
            eg = lwork.tile([P, P], f32, tag="eg")
            nc.vector.tensor_copy(eg[:], g[:])
            scr = lwork.tile([P, P], f32, tag="scr")
            nc.vector.tensor_tensor(scr[:], eg[:], im_t[:], ALU.mult)
            d2 = small.tile([P, 1], f32, tag="d2")
            nc.vector.tensor_reduce(d2[:], scr[:], axis=AX.X, op=ALU.add)
            lnv = small.tile([P, 1], f32, tag="lnv")
            nc.scalar.activation(lnv[:], d2[:], ACTF.Ln, scale=TEMP)
            rnx = small.tile([P, 1], f32, tag="rnx")
            nc.scalar.activation(rnx[:], lnv[:], ACTF.Exp, scale=-0.5)
            drn = lwork.tile([P, P], f32, tag="drn")
            nc.vector.tensor_scalar_mul(drn[:], im_t[:], rnx[:])
            h_ps = rpsum.tile([P, P], f32, tag="r")
            nc.tensor.matmul(h_ps[:], eg[:], drn[:], start=True, stop=True)
            lt = lwork.tile([P, P], f32, tag="lt")
            nc.vector.tensor_scalar_mul(lt[:], h_ps[:], rnx[:])

            xt = lwork.tile([P, P], f32, tag="xt")
            nc.scalar.activation(xt[:], lt[:], ACTF.Exp)
            xm = lwork.tile([P, P], f32, tag="xm")
            nc.vector.tensor_tensor(xm[:], xt[:], m0_t[:], ALU.mult)
            st = small.tile([P, 1], f32, tag="st")
            nc.vector.tensor_reduce(st[:], xm[:], axis=AX.X, op=ALU.add)
            nc.scalar.activation(ldcols[:, t:t + 1], st[:], ACTF.Ln)
            lp = lwork.tile([P, P], f32, tag="lp")
            nc.vector.tensor_tensor(lp[:], lt[:], pm_t[:], ALU.mult)
            nc.vector.tensor_reduce(t1cols[:, t:t + 1], lp[:], axis=AX.X,
                                    op=ALU.add)

        # ---- final weighted reduction ----
        z1 = work.tile([P, T], f32, tag="z1")
        nc.vector.tensor_tensor(z1[:], t1cols[:], w1_t[:], ALU.mult)
        z2 = work.tile([P, T], f32, tag="z2")
        nc.vector.tensor_tensor(z2[:], ldcols[:], w2_t[:], ALU.mult)
        zs = work.tile([P, T], f32, tag="zs")
        nc.vector.tensor_add(zs[:], z1[:], z2[:])
        zc = work.tile([P, 1], f32, tag="zc")
        nc.vector.tensor_reduce(zc[:], zs[:], axis=AX.X, op=ALU.add)

        ones = work.tile([P, 1], f32, tag="ones")
        nc.vector.memset(ones[:], 1.0)
        tot_ps = gpsum.tile([1, 1], f32, tag="g")
        nc.tensor.matmul(tot_ps[:, :], zc[:], ones[:], start=True, stop=True)
        tot = work.tile([1, 1], f32, tag="tot")
        nc.vector.tensor_copy(tot[:], tot_ps[:, :])
        nc.sync.dma_start(out_dram[:, :], tot[:])

    nc.compile()
    return nc


def _ensure_axon_hooks():
    """Provide antenv.axon_hooks if the image lacks it (NTFF profiling shim).

    Mirrors trn_agent_boot.trn_boot: the hook drives NRT profiling via the
    libaxon_pjrt.so C ABI.  If anything is missing we register a None hook,
    which makes bass_utils skip tracing gracefully instead of crashing.
    """
    try:
        import antenv.axon_hooks  # noqa: F401
        return
    except ImportError:
        pass
    import contextlib
    import ctypes
    import types

    import antenv

    hook = None
    so_path = "/opt/axon/libaxon_pjrt.so"
    try:
        lib = ctypes.CDLL(so_path)
        if hasattr(lib, "axon_start_nrt_profile"):
            lib.axon_start_nrt_profile.argtypes = [
                ctypes.POINTER(ctypes.c_int64), ctypes.c_size_t]
            lib.axon_start_nrt_profile.restype = ctypes.c_int64
            lib.axon_stop_nrt_profile.argtypes = [ctypes.c_char_p]
            lib.axon_stop_nrt_profile.restype = ctypes.c_int64

            @contextlib.contextmanager
            def _hook(output_dir, device_ids):
                import jax
                jax.devices()
                if device_ids:
                    ids = (ctypes.c_int64 * len(device_ids))(*device_ids)
                    rc = lib.axon_start_nrt_profile(ids, len(device_ids))
                else:
                    rc = lib.axon_start_nrt_profile(None, 0)
                if rc != 0:
                    raise RuntimeError(f"axon_start_nrt_profile rc={rc}")
                try:
                    yield
                finally:
                    n = lib.axon_stop_nrt_profile(str(output_dir).encode())
                    print(f"profile: {n} file(s) written to {output_dir}",
                          file=sys.stderr)

            hook = _hook
    except OSError:
        pass

    mod = types.ModuleType("antenv.axon_hooks")
    state = {"hook": hook}
    mod.get_axon_ntff_profile_hook = lambda: state["hook"]
    mod.set_axon_ntff_profile_hook = lambda h: state.__setitem__("hook", h)
    sys.modules["antenv.axon_hooks"] = mod
    antenv.axon_hooks = mod


def kernel(**inputs):
    global _compiled, LAST_RESULTS
    from concourse import bass_utils

    feats = np.ascontiguousarray(
        np.asarray(inputs["features"], dtype=np.float32).reshape(B * M, D))

    if _compiled is None:
        _compiled = (_build(), _host_consts())
    nc, consts = _compiled

    in_maps = []
    for k in range(NCORES):
        im = dict(consts)
        im["f"] = feats[k * ROWS:(k + 1) * ROWS]
        in_maps.append(im)

    trace = bool(os.environ.get("BASS_TRACE"))
    if trace:
        _ensure_axon_hooks()
    try:
        res = bass_utils.run_bass_kernel_spmd(
            nc, in_maps, core_ids=list(range(NCORES)), trace=trace)
    except Exception:
        if not trace:
            raise
        # Tracing plumbing failed; rerun untraced so the result is still valid.
        os.environ["BASS_NEVER_TRACE"] = "1"
        try:
            res = bass_utils.run_bass_kernel_spmd(
                nc, in_maps, core_ids=list(range(NCORES)), trace=False)
        finally:
            del os.environ["BASS_NEVER_TRACE"]
    LAST_RESULTS = res
    total = float(np.sum([np.float64(r["out"][0, 0]) for r in res.results]))
    return np.array(total, dtype=np.float32)


# revision 16
# speedup vs baseline: 1.4156x; 1.1941x over previous
"""Trainium2 Bass kernel for nn_BRCLoss (supervised-contrastive style loss).

Math (per batch sample b, matching the jax reference):
    f = features[b].reshape(24, 4096); fhat = f / ||f||_row
    logits = (fhat @ fhat.T) / 0.1                       # [24, 24]
    exp_logits = exp(logits) * (1 - I)
    log_prob = logits - log(exp_logits.sum(-1))
    mlpp = (mask * log_prob).sum(-1) / (mask.sum(-1) + 1e-6)
    loss = sum_b mean_m(-0.1 * mlpp) / 512               # scalar

`outputs` / `targets` are unused by the reference; only `features`
[512, 2, 12, 4096] f32 matters.  Pure data parallel: 64 samples per core.

Per-core kernel:
  - 13 tiles of [120 rows, 4096] (5 samples each; the last tile re-reads the
    trailing 120 rows, the duplicated sample is zero-weighted).
  - For each tile: PE-transposes 32 f32 chunks [120,128] -> PSUM [128,120],
    downcast-copies to bf16 SBUF (alternating DVE/ACT), then 32 accumulating
    [128,120]x[128,120] bf16 matmuls -> raw per-tile Gram [120,120] (block
    diagonal holds the 5 per-sample 24x24 Grams).
  - Epilogue: row norms come off the Gram diagonal (identity mask + reduce),
    normalization + 1/temperature are applied as a rank-1 outer product of
    rsqrt-norms via K=1 matmuls, then masked exp/log-sum and a weighted
    reduction with host-precomputed masks/weights fold the whole loss
    (including mean/positive-count/validity bookkeeping) into one dot product.
Host sums the 8 per-core scalars.
"""

import os
import sys

import numpy as np

if "/opt/trn_rl_repo" not in sys.path:
    sys.path.insert(0, "/opt/trn_rl_repo")

# Problem constants (hardcoded; kernel.py must be self-contained).
B = 512
NV = 2
NCLS = 12
D = 4096
M = NV * NCLS              # 24 anchor rows per sample
NCORES = 8
SPC = B // NCORES          # 64 samples per core
ROWS = SPC * M             # 1536 feature rows per core
P = 120                    # rows per tile (5 samples)
G5 = P // M                # samples per tile
T = 13                     # tiles per core (12 aligned + 1 overlapping tail)
CH = 128                   # contraction chunk (PE partition limit)
NCH = D // CH              # 32 chunks
QUAD = 4                   # transposed chunks packed per PSUM bank
NQ = NCH // QUAD
TEMP = 0.1
EPS_POS = 1e-6

_ROW_STARTS = [P * t for t in range(T - 1)] + [ROWS - P]

_compiled = None           # (nc, const_in_map)
LAST_RESULTS = None        # BassKernelResults of the most recent run


def _host_consts():
    """Masks/weights shared by every core (the per-core sample grid is identical)."""
    i = np.arange(NCLS)
    graph = (np.abs(i[:, None] - i[None, :]) <= 1).astype(np.float32)   # [12,12]
    eye24 = np.eye(M, dtype=np.float32)
    mask24 = np.tile(graph, (NV, NV)) * (1.0 - eye24)                    # positives
    blk = np.kron(np.eye(G5, dtype=np.float32), np.ones((M, M), np.float32))
    m0 = (blk * (1.0 - np.eye(P, dtype=np.float32))).astype(np.float32)  # denom mask
    pm = np.kron(np.eye(G5, dtype=np.float32), mask24).astype(np.float32)
    im = (TEMP * np.eye(P)).astype(np.float32)   # folds the 1/temperature scale
    msum = np.tile(mask24.sum(1), G5).astype(np.float64)                 # [120], 3 or 5
    alpha = -TEMP / ((msum + EPS_POS) * M * B)                           # per-row weight
    valid = np.ones((P, T), np.float64)
    valid[:M, T - 1] = 0.0   # last tile re-reads sample 59 -> zero weight
    w1 = (alpha[:, None] * valid).astype(np.float32)
    w2 = ((-alpha * msum)[:, None] * valid).astype(np.float32)
    return {"m0": m0, "pm": pm, "im": im, "w1": w1, "w2": w2}


def _build():
    from contextlib import ExitStack

    from concourse import bacc, bass, masks, mybir, tile

    f32 = mybir.dt.float32
    bf16 = mybir.dt.bfloat16
    AX = mybir.AxisListType
    ALU = mybir.AluOpType
    ACTF = mybir.ActivationFunctionType

    nc = bacc.Bacc("TRN2", target_bir_lowering=False, debug=False,
                   num_devices=NCORES)

    f_dram = nc.dram_tensor("f", (ROWS, D), f32, kind="ExternalInput")
    m0_dram = nc.dram_tensor("m0", (P, P), f32, kind="ExternalInput")
    pm_dram = nc.dram_tensor("pm", (P, P), f32, kind="ExternalInput")
    im_dram = nc.dram_tensor("im", (P, P), f32, kind="ExternalInput")
    w1_dram = nc.dram_tensor("w1", (P, T), f32, kind="ExternalInput")
    w2_dram = nc.dram_tensor("w2", (P, T), f32, kind="ExternalInput")
    out_dram = nc.dram_tensor("out", (1, 1), f32, kind="ExternalOutput")

    DSPLIT = 4                 # DMAs per feature tile
    DCOLS = D // DSPLIT

    with ExitStack() as ctx:
        tc = ctx.enter_context(tile.TileContext(nc))
        consts = ctx.enter_context(tc.tile_pool(name="consts", bufs=1))
        fpool = ctx.enter_context(tc.tile_pool(name="fpool", bufs=3))
        tcpool = ctx.enter_context(tc.tile_pool(name="tcpool", bufs=6))
        work = ctx.enter_context(tc.tile_pool(name="work", bufs=1))
        lwork = ctx.enter_context(tc.tile_pool(name="lwork", bufs=2))
        small = ctx.enter_context(tc.tile_pool(name="small", bufs=2))
        tpsum = ctx.enter_context(
            tc.tile_pool(name="tpsum", bufs=3, space=bass.MemorySpace.PSUM))
        gpsum = ctx.enter_context(
            tc.tile_pool(name="gpsum", bufs=2, space=bass.MemorySpace.PSUM))
        rpsum = ctx.enter_context(
            tc.tile_pool(name="rpsum", bufs=2, space=bass.MemorySpace.PSUM))

        # Feature loads lead the program so the sync-ring FIFO starts streaming
        # them immediately; constants ride the scalar HWDGE ring in parallel.
        ftiles = []
        for t in range(T):
            r0 = _ROW_STARTS[t]
            ft = fpool.tile([P, D], f32, tag="f")
            if t < 3:
                for q in range(DSPLIT):
                    nc.sync.dma_start(ft[:, q * DCOLS:(q + 1) * DCOLS],
                                      f_dram[r0:r0 + P, q * DCOLS:(q + 1) * DCOLS])
            ftiles.append(ft)

        ident = consts.tile([128, 128], f32, tag="ident")
        masks.make_identity(nc, ident[:])
        m0_t = consts.tile([P, P], f32, tag="m0")
        pm_t = consts.tile([P, P], f32, tag="pm")
        im_t = consts.tile([P, P], f32, tag="im")
        w1_t = consts.tile([P, T], f32, tag="w1")
        w2_t = consts.tile([P, T], f32, tag="w2")
        nc.scalar.dma_start(m0_t[:], m0_dram[:, :])
        nc.scalar.dma_start(pm_t[:], pm_dram[:, :])
        nc.scalar.dma_start(im_t[:], im_dram[:, :])
        nc.scalar.dma_start(w1_t[:], w1_dram[:, :])
        nc.scalar.dma_start(w2_t[:], w2_dram[:, :])

        # Preload the exp/ln activation table set while DMA streams.
        warm = consts.tile([1, 2], f32, tag="warm")
        nc.vector.memset(warm[:], 1.0)
        nc.scalar.activation(warm[:, 1:2], warm[:, 0:1], ACTF.Exp)

        t1cols = work.tile([P, T], f32, tag="t1cols")   # sum(mask*logits) per tile
        scols = work.tile([P, T], f32, tag="scols")     # softmax denominators
        d2cols = work.tile([P, T], f32, tag="d2cols")   # 0.1 * squared row norms
        rnxcols = work.tile([P, T], f32, tag="rnxcols")  # (0.1*d2)^-0.5
        egpool = ctx.enter_context(tc.tile_pool(name="egpool", bufs=8))

        # Activation-table discipline: Ln and Exp live in different ACT table
        # sets (~1.3us per switch), so the norm transcendentals are batched per
        # group of tiles and the denominator Ln once at the end.
        GROUPS = [(0, 5), (5, 10), (10, T)]
        egs = {}

        def tile_gram(t):
            ft = ftiles[t]
            if t >= 3:
                r0 = _ROW_STARTS[t]
                for q in range(DSPLIT):
                    nc.sync.dma_start(ft[:, q * DCOLS:(q + 1) * DCOLS],
                                      f_dram[r0:r0 + P, q * DCOLS:(q + 1) * DCOLS])
            g = gpsum.tile([P, P], f32, tag="g")
            for q in range(NQ):
                tp = tpsum.tile([128, QUAD * P], f32, tag="tp")
                for j in range(QUAD):
                    c = q * QUAD + j
                    nc.tensor.transpose(
                        tp[:, j * P:(j + 1) * P],
                        ft[:, c * CH:(c + 1) * CH],
                        ident[:P, :P],
                    )
                tcs = tcpool.tile([128, QUAD * P], bf16, tag="tc")
                if q % 2 == 0:
                    nc.vector.tensor_copy(tcs[:], tp[:])
                else:
                    nc.scalar.copy(tcs[:], tp[:])
                for j in range(QUAD):
                    c = q * QUAD + j
                    sl = tcs[:, j * P:(j + 1) * P]
                    nc.tensor.matmul(g[:], sl, sl,
                                     start=(c == 0), stop=(c == NCH - 1))
            eg = egpool.tile([P, P], f32, tag="eg")
            nc.vector.tensor_copy(eg[:], g[:])
            egs[t] = eg
            # d2cols[:, t] = 0.1 * diag(G)  (im_t is pre-scaled by TEMP)
            scr = lwork.tile([P, P], f32, tag="scr")
            nc.vector.tensor_tensor(scr[:], eg[:], im_t[:], ALU.mult)
            nc.vector.tensor_reduce(d2cols[:, t:t + 1], scr[:], axis=AX.X,
                                    op=ALU.add)

        def tile_softmax(t):
            # logits L = diag(rnx) @ G @ diag(rnx); the column scaling runs on
            # the PE as G @ diag(rnx) (G is symmetric so lhsT=G is G^T), the
            # row scaling as a per-partition tensor_scalar.
            eg = egs.pop(t)
            rnx = rnxcols[:, t:t + 1]
            drn = lwork.tile([P, P], f32, tag="drn")
            nc.vector.tensor_scalar(drn[:], im_t[:], rnx, 1.0 / TEMP,
                                    op0=ALU.mult, op1=ALU.mult)
            h_ps = rpsum.tile([P, P], f32, tag="r")
            nc.tensor.matmul(h_ps[:], eg[:], drn[:], start=True, stop=True)
            lt = lwork.tile([P, P], f32, tag="lt")
            nc.vector.tensor_scalar_mul(lt[:], h_ps[:], rnx)
            xt = lwork.tile([P, P], f32, tag="xt")
            nc.scalar.activation(xt[:], lt[:], ACTF.Exp)
            xm = lwork.tile([P, P], f32, tag="xm")
            nc.vector.tensor_tensor(xm[:], xt[:], m0_t[:], ALU.mult)
            nc.vector.tensor_reduce(scols[:, t:t + 1], xm[:], axis=AX.X,
                                    op=ALU.add)
            lp = lwork.tile([P, P], f32, tag="lp")
            nc.vector.tensor_tensor(lp[:], lt[:], pm_t[:], ALU.mult)
            nc.vector.tensor_reduce(t1cols[:, t:t + 1], lp[:], axis=AX.X,
                                    op=ALU.add)

        for g0, g1 in GROUPS:
            for t in range(g0, g1):
                tile_gram(t)
            lnv = work.tile([P, T], f32, tag="lnv")
            nc.scalar.activation(lnv[:, g0:g1], d2cols[:, g0:g1], ACTF.Ln)
            nc.scalar.activation(rnxcols[:, g0:g1], lnv[:, g0:g1], ACTF.Exp,
                                 scale=-0.5)
            for t in range(g0, g1):
                tile_softmax(t)

        # ---- final weighted reduction ----
        ld = work.tile([P, T], f32, tag="ld")
        nc.scalar.activation(ld[:], scols[:], ACTF.Ln)
        z1 = work.tile([P, T], f32, tag="z1")
        nc.vector.tensor_tensor(z1[:], t1cols[:], w1_t[:], ALU.mult)
        z2 = work.tile([P, T], f32, tag="z2")
        nc.vector.tensor_tensor(z2[:], ld[:], w2_t[:], ALU.mult)
        zs = work.tile([P, T], f32, tag="zs")
        nc.vector.tensor_add(zs[:], z1[:], z2[:])
        zc = work.tile([P, 1], f32, tag="zc")
        nc.vector.tensor_reduce(zc[:], zs[:], axis=AX.X, op=ALU.add)

        ones = work.tile([P, 1], f32, tag="ones")
        nc.vector.memset(ones[:], 1.0)
        tot_ps = gpsum.tile([1, 1], f32, tag="g")
        nc.tensor.matmul(tot_ps[:, :], zc[:], ones[:], start=True, stop=True)
        tot = work.tile([1, 1], f32, tag="tot")
        nc.vector.tensor_copy(tot[:], tot_ps[:, :])
        nc.sync.dma_start(out_dram[:, :], tot[:])

    nc.compile()
    return nc


def _ensure_axon_hooks():
    """Provide antenv.axon_hooks if the image lacks it (NTFF profiling shim).

    Mirrors trn_agent_boot.trn_boot: the hook drives NRT profiling via the
    libaxon_pjrt.so C ABI.  If anything is missing we register a None hook,
    which makes bass_utils skip tracing gracefully instead of crashing.
    """
    try:
        import antenv.axon_hooks  # noqa: F401
        return
    except ImportError:
        pass
    import contextlib
    import ctypes
    import types

    import antenv

    hook = None
    so_path = "/opt/axon/libaxon_pjrt.so"
    try:
        lib = ctypes.CDLL(so_path)
        if hasattr(lib, "axon_start_nrt_profile"):
            lib.axon_start_nrt_profile.argtypes = [
                ctypes.POINTER(ctypes.c_int64), ctypes.c_size_t]
            lib.axon_start_nrt_profile.restype = ctypes.c_int64
            lib.axon_stop_nrt_profile.argtypes = [ctypes.c_char_p]
            lib.axon_stop_nrt_profile.restype = ctypes.c_int64

            @contextlib.contextmanager
            def _hook(output_dir, device_ids):
                import jax
                jax.devices()
                if device_ids:
                    ids = (ctypes.c_int64 * len(device_ids))(*device_ids)
                    rc = lib.axon_start_nrt_profile(ids, len(device_ids))
                else:
                    rc = lib.axon_start_nrt_profile(None, 0)
                if rc != 0:
                    raise RuntimeError(f"axon_start_nrt_profile rc={rc}")
                try:
                    yield
                finally:
                    n = lib.axon_stop_nrt_profile(str(output_dir).encode())
                    print(f"profile: {n} file(s) written to {output_dir}",
                          file=sys.stderr)

            hook = _hook
    except OSError:
        pass

    mod = types.ModuleType("antenv.axon_hooks")
    state = {"hook": hook}
    mod.get_axon_ntff_profile_hook = lambda: state["hook"]
    mod.set_axon_ntff_profile_hook = lambda h: state.__setitem__("hook", h)
    sys.modules["antenv.axon_hooks"] = mod
    antenv.axon_hooks = mod


def kernel(**inputs):
    global _compiled, LAST_RESULTS
    from concourse import bass_utils

    feats = np.ascontiguousarray(
        np.asarray(inputs["features"], dtype=np.float32).reshape(B * M, D))

    if _compiled is None:
        _compiled = (_build(), _host_consts())
    nc, consts = _compiled

    in_maps = []
    for k in range(NCORES):
        im = dict(consts)
        im["f"] = feats[k * ROWS:(k + 1) * ROWS]
        in_maps.append(im)

    trace = bool(os.environ.get("BASS_TRACE"))
    if trace:
        _ensure_axon_hooks()
    try:
        res = bass_utils.run_bass_kernel_spmd(
            nc, in_maps, core_ids=list(range(NCORES)), trace=trace)
    except Exception:
        if not trace:
            raise
        # Tracing plumbing failed; rerun untraced so the result is still valid.
        os.environ["BASS_NEVER_TRACE"] = "1"
        try:
            res = bass_utils.run_bass_kernel_spmd(
                nc, in_maps, core_ids=list(range(NCORES)), trace=False)
        finally:
            del os.environ["BASS_NEVER_TRACE"]
    LAST_RESULTS = res
    total = float(np.sum([np.float64(r["out"][0, 0]) for r in res.results]))
    return np.array(total, dtype=np.float32)


# revision 17
# speedup vs baseline: 1.7344x; 1.2252x over previous
"""Trainium2 Bass kernel for nn_BRCLoss (supervised-contrastive style loss).

Math (per batch sample b, matching the jax reference):
    f = features[b].reshape(24, 4096); fhat = f / ||f||_row
    logits = (fhat @ fhat.T) / 0.1                       # [24, 24]
    exp_logits = exp(logits) * (1 - I)
    log_prob = logits - log(exp_logits.sum(-1))
    mlpp = (mask * log_prob).sum(-1) / (mask.sum(-1) + 1e-6)
    loss = sum_b mean_m(-0.1 * mlpp) / 512               # scalar

`outputs` / `targets` are unused by the reference; only `features`
[512, 2, 12, 4096] f32 matters.  Pure data parallel: 64 samples per core.

Per-core kernel:
  - 13 tiles of [120 rows, 4096] (5 samples each; the last tile re-reads the
    trailing 120 rows, the duplicated sample is zero-weighted).
  - For each tile: PE-transposes 32 f32 chunks [120,128] -> PSUM [128,120],
    downcast-copies to bf16 SBUF (alternating DVE/ACT), then 32 accumulating
    [128,120]x[128,120] bf16 matmuls -> raw per-tile Gram [120,120] (block
    diagonal holds the 5 per-sample 24x24 Grams).
  - Epilogue: row norms come off the Gram diagonal (identity mask + reduce),
    normalization + 1/temperature are applied as a rank-1 outer product of
    rsqrt-norms via K=1 matmuls, then masked exp/log-sum and a weighted
    reduction with host-precomputed masks/weights fold the whole loss
    (including mean/positive-count/validity bookkeeping) into one dot product.
Host sums the 8 per-core scalars.
"""

import os
import sys

import numpy as np

if "/opt/trn_rl_repo" not in sys.path:
    sys.path.insert(0, "/opt/trn_rl_repo")

# Problem constants (hardcoded; kernel.py must be self-contained).
B = 512
NV = 2
NCLS = 12
D = 4096
M = NV * NCLS              # 24 anchor rows per sample
NCORES = 8
SPC = B // NCORES          # 64 samples per core
ROWS = SPC * M             # 1536 feature rows per core
P = 120                    # rows per tile (5 samples)
G5 = P // M                # samples per tile
T = 13                     # tiles per core (12 aligned + 1 overlapping tail)
CH = 128                   # contraction chunk (PE partition limit)
NCH = D // CH              # 32 chunks
QUAD = 4                   # transposed chunks packed per PSUM bank
NQ = NCH // QUAD
TEMP = 0.1
EPS_POS = 1e-6

_ROW_STARTS = [P * t for t in range(T - 1)] + [ROWS - P]

_compiled = None           # (nc, const_in_map)
LAST_RESULTS = None        # BassKernelResults of the most recent run


def _host_consts():
    """Masks/weights shared by every core (the per-core sample grid is identical)."""
    i = np.arange(NCLS)
    graph = (np.abs(i[:, None] - i[None, :]) <= 1).astype(np.float32)   # [12,12]
    eye24 = np.eye(M, dtype=np.float32)
    mask24 = np.tile(graph, (NV, NV)) * (1.0 - eye24)                    # positives
    blk = np.kron(np.eye(G5, dtype=np.float32), np.ones((M, M), np.float32))
    m0 = (blk * (1.0 - np.eye(P, dtype=np.float32))).astype(np.float32)  # denom mask
    pm = np.kron(np.eye(G5, dtype=np.float32), mask24).astype(np.float32)
    im = (TEMP * np.eye(P)).astype(np.float32)   # folds the 1/temperature scale
    msum = np.tile(mask24.sum(1), G5).astype(np.float64)                 # [120], 3 or 5
    alpha = -TEMP / ((msum + EPS_POS) * M * B)                           # per-row weight
    valid = np.ones((P, T), np.float64)
    valid[:M, T - 1] = 0.0   # last tile re-reads sample 59 -> zero weight
    w1 = (alpha[:, None] * valid).astype(np.float32)
    w2 = ((-alpha * msum)[:, None] * valid).astype(np.float32)
    return {"m0": m0, "pm": pm, "im": im, "w1": w1, "w2": w2}


def _build():
    from contextlib import ExitStack

    from concourse import bacc, bass, masks, mybir, tile

    f32 = mybir.dt.float32
    bf16 = mybir.dt.bfloat16
    AX = mybir.AxisListType
    ALU = mybir.AluOpType
    ACTF = mybir.ActivationFunctionType

    nc = bacc.Bacc("TRN2", target_bir_lowering=False, debug=False,
                   num_devices=NCORES)

    f_dram = nc.dram_tensor("f", (ROWS, D), f32, kind="ExternalInput")
    m0_dram = nc.dram_tensor("m0", (P, P), f32, kind="ExternalInput")
    pm_dram = nc.dram_tensor("pm", (P, P), f32, kind="ExternalInput")
    im_dram = nc.dram_tensor("im", (P, P), f32, kind="ExternalInput")
    w1_dram = nc.dram_tensor("w1", (P, T), f32, kind="ExternalInput")
    w2_dram = nc.dram_tensor("w2", (P, T), f32, kind="ExternalInput")
    out_dram = nc.dram_tensor("out", (1, 1), f32, kind="ExternalOutput")

    DSPLIT = 4                 # DMAs per feature tile
    DCOLS = D // DSPLIT

    with ExitStack() as ctx:
        tc = ctx.enter_context(tile.TileContext(nc))
        consts = ctx.enter_context(tc.tile_pool(name="consts", bufs=1))
        fpool = ctx.enter_context(tc.tile_pool(name="fpool", bufs=4))
        tcpool = ctx.enter_context(tc.tile_pool(name="tcpool", bufs=6))
        work = ctx.enter_context(tc.tile_pool(name="work", bufs=1))
        lwork = ctx.enter_context(tc.tile_pool(name="lwork", bufs=2))
        small = ctx.enter_context(tc.tile_pool(name="small", bufs=2))
        tpsum = ctx.enter_context(
            tc.tile_pool(name="tpsum", bufs=3, space=bass.MemorySpace.PSUM))
        gpsum = ctx.enter_context(
            tc.tile_pool(name="gpsum", bufs=2, space=bass.MemorySpace.PSUM))
        rpsum = ctx.enter_context(
            tc.tile_pool(name="rpsum", bufs=2, space=bass.MemorySpace.PSUM))

        # Feature loads lead the program so the sync-ring FIFO starts streaming
        # them immediately; constants ride the scalar HWDGE ring in parallel.
        ftiles = []
        for t in range(T):
            r0 = _ROW_STARTS[t]
            ft = fpool.tile([P, D], bf16, tag="f")
            if t < 3:
                for q in range(DSPLIT):
                    nc.gpsimd.dma_start(ft[:, q * DCOLS:(q + 1) * DCOLS],
                                        f_dram[r0:r0 + P, q * DCOLS:(q + 1) * DCOLS])
            ftiles.append(ft)

        ident = consts.tile([128, 128], f32, tag="ident")
        masks.make_identity(nc, ident[:])
        identb = consts.tile([128, 128], bf16, tag="identb")
        masks.make_identity(nc, identb[:])
        m0_t = consts.tile([P, P], f32, tag="m0")
        pm_t = consts.tile([P, P], f32, tag="pm")
        im_t = consts.tile([P, P], f32, tag="im")
        w1_t = consts.tile([P, T], f32, tag="w1")
        w2_t = consts.tile([P, T], f32, tag="w2")
        nc.scalar.dma_start(m0_t[:], m0_dram[:, :])
        nc.scalar.dma_start(pm_t[:], pm_dram[:, :])
        nc.scalar.dma_start(im_t[:], im_dram[:, :])
        nc.scalar.dma_start(w1_t[:], w1_dram[:, :])
        nc.scalar.dma_start(w2_t[:], w2_dram[:, :])

        # Preload the exp/ln activation table set while DMA streams.
        warm = consts.tile([1, 2], f32, tag="warm")
        nc.vector.memset(warm[:], 1.0)
        nc.scalar.activation(warm[:, 1:2], warm[:, 0:1], ACTF.Exp)

        t1cols = work.tile([P, T], f32, tag="t1cols")   # sum(mask*logits) per tile
        scols = work.tile([P, T], f32, tag="scols")     # softmax denominators
        d2cols = work.tile([P, T], f32, tag="d2cols")   # 0.1 * squared row norms
        rnxcols = work.tile([P, T], f32, tag="rnxcols")  # (0.1*d2)^-0.5
        egpool = ctx.enter_context(tc.tile_pool(name="egpool", bufs=8))

        # Activation-table discipline: Ln and Exp live in different ACT table
        # sets (~1.3us per switch), so the norm transcendentals are batched per
        # group of tiles and the denominator Ln once at the end.
        GROUPS = [(0, 5), (5, 10), (10, T)]
        egs = {}

        def tile_gram(t):
            ft = ftiles[t]
            if t >= 3:
                r0 = _ROW_STARTS[t]
                for q in range(DSPLIT):
                    nc.gpsimd.dma_start(ft[:, q * DCOLS:(q + 1) * DCOLS],
                                        f_dram[r0:r0 + P, q * DCOLS:(q + 1) * DCOLS])
            g = gpsum.tile([P, P], f32, tag="g")
            for q in range(NQ):
                tp = tpsum.tile([128, QUAD * P], bf16, tag="tp")
                for j in range(QUAD):
                    c = q * QUAD + j
                    nc.tensor.transpose(
                        tp[:, j * P:(j + 1) * P],
                        ft[:, c * CH:(c + 1) * CH],
                        identb[:P, :P],
                    )
                tcs = tcpool.tile([128, QUAD * P], bf16, tag="tc")
                if q % 2 == 0:
                    nc.vector.tensor_copy(tcs[:], tp[:])
                else:
                    nc.scalar.copy(tcs[:], tp[:])
                for j in range(QUAD):
                    c = q * QUAD + j
                    sl = tcs[:, j * P:(j + 1) * P]
                    nc.tensor.matmul(g[:], sl, sl,
                                     start=(c == 0), stop=(c == NCH - 1))
            eg = egpool.tile([P, P], f32, tag="eg")
            nc.vector.tensor_copy(eg[:], g[:])
            egs[t] = eg
            # d2cols[:, t] = 0.1 * diag(G)  (im_t is pre-scaled by TEMP)
            scr = lwork.tile([P, P], f32, tag="scr")
            nc.vector.tensor_tensor(scr[:], eg[:], im_t[:], ALU.mult)
            nc.vector.tensor_reduce(d2cols[:, t:t + 1], scr[:], axis=AX.X,
                                    op=ALU.add)

        def tile_softmax(t):
            # logits L = diag(rnx) @ G @ diag(rnx); the column scaling runs on
            # the PE as G @ diag(rnx) (G is symmetric so lhsT=G is G^T), the
            # row scaling as a per-partition tensor_scalar.
            eg = egs.pop(t)
            rnx = rnxcols[:, t:t + 1]
            drn = lwork.tile([P, P], f32, tag="drn")
            nc.vector.tensor_scalar(drn[:], im_t[:], rnx, 1.0 / TEMP,
                                    op0=ALU.mult, op1=ALU.mult)
            h_ps = rpsum.tile([P, P], f32, tag="r")
            nc.tensor.matmul(h_ps[:], eg[:], drn[:], start=True, stop=True)
            lt = lwork.tile([P, P], f32, tag="lt")
            nc.vector.tensor_scalar_mul(lt[:], h_ps[:], rnx)
            xt = lwork.tile([P, P], f32, tag="xt")
            nc.scalar.activation(xt[:], lt[:], ACTF.Exp)
            xm = lwork.tile([P, P], f32, tag="xm")
            nc.vector.tensor_tensor(xm[:], xt[:], m0_t[:], ALU.mult)
            nc.vector.tensor_reduce(scols[:, t:t + 1], xm[:], axis=AX.X,
                                    op=ALU.add)
            lp = lwork.tile([P, P], f32, tag="lp")
            nc.vector.tensor_tensor(lp[:], lt[:], pm_t[:], ALU.mult)
            nc.vector.tensor_reduce(t1cols[:, t:t + 1], lp[:], axis=AX.X,
                                    op=ALU.add)

        for g0, g1 in GROUPS:
            for t in range(g0, g1):
                tile_gram(t)
            lnv = work.tile([P, T], f32, tag="lnv")
            nc.scalar.activation(lnv[:, g0:g1], d2cols[:, g0:g1], ACTF.Ln)
            nc.scalar.activation(rnxcols[:, g0:g1], lnv[:, g0:g1], ACTF.Exp,
                                 scale=-0.5)
            for t in range(g0, g1):
                tile_softmax(t)

        # ---- final weighted reduction ----
        ld = work.tile([P, T], f32, tag="ld")
        nc.scalar.activation(ld[:], scols[:], ACTF.Ln)
        z1 = work.tile([P, T], f32, tag="z1")
        nc.vector.tensor_tensor(z1[:], t1cols[:], w1_t[:], ALU.mult)
        z2 = work.tile([P, T], f32, tag="z2")
        nc.vector.tensor_tensor(z2[:], ld[:], w2_t[:], ALU.mult)
        zs = work.tile([P, T], f32, tag="zs")
        nc.vector.tensor_add(zs[:], z1[:], z2[:])
        zc = work.tile([P, 1], f32, tag="zc")
        nc.vector.tensor_reduce(zc[:], zs[:], axis=AX.X, op=ALU.add)

        ones = work.tile([P, 1], f32, tag="ones")
        nc.vector.memset(ones[:], 1.0)
        tot_ps = gpsum.tile([1, 1], f32, tag="g")
        nc.tensor.matmul(tot_ps[:, :], zc[:], ones[:], start=True, stop=True)
        tot = work.tile([1, 1], f32, tag="tot")
        nc.vector.tensor_copy(tot[:], tot_ps[:, :])
        nc.sync.dma_start(out_dram[:, :], tot[:])

    nc.compile()
    return nc


def _ensure_axon_hooks():
    """Provide antenv.axon_hooks if the image lacks it (NTFF profiling shim).

    Mirrors trn_agent_boot.trn_boot: the hook drives NRT profiling via the
    libaxon_pjrt.so C ABI.  If anything is missing we register a None hook,
    which makes bass_utils skip tracing gracefully instead of crashing.
    """
    try:
        import antenv.axon_hooks  # noqa: F401
        return
    except ImportError:
        pass
    import contextlib
    import ctypes
    import types

    import antenv

    hook = None
    so_path = "/opt/axon/libaxon_pjrt.so"
    try:
        lib = ctypes.CDLL(so_path)
        if hasattr(lib, "axon_start_nrt_profile"):
            lib.axon_start_nrt_profile.argtypes = [
                ctypes.POINTER(ctypes.c_int64), ctypes.c_size_t]
            lib.axon_start_nrt_profile.restype = ctypes.c_int64
            lib.axon_stop_nrt_profile.argtypes = [ctypes.c_char_p]
            lib.axon_stop_nrt_profile.restype = ctypes.c_int64

            @contextlib.contextmanager
            def _hook(output_dir, device_ids):
                import jax
                jax.devices()
                if device_ids:
                    ids = (ctypes.c_int64 * len(device_ids))(*device_ids)
                    rc = lib.axon_start_nrt_profile(ids, len(device_ids))
                else:
                    rc = lib.axon_start_nrt_profile(None, 0)
                if rc != 0:
                    raise RuntimeError(f"axon_start_nrt_profile rc={rc}")
                try:
                    yield
                finally:
                    n = lib.axon_stop_nrt_profile(str(output_dir).encode())
                    print(f"profile: {n} file(s) written to {output_dir}",
                          file=sys.stderr)

            hook = _hook
    except OSError:
        pass

    mod = types.ModuleType("antenv.axon_hooks")
    state = {"hook": hook}
    mod.get_axon_ntff_profile_hook = lambda: state["hook"]
    mod.set_axon_ntff_profile_hook = lambda h: state.__setitem__("hook", h)
    sys.modules["antenv.axon_hooks"] = mod
    antenv.axon_hooks = mod


def kernel(**inputs):
    global _compiled, LAST_RESULTS
    from concourse import bass_utils

    feats = np.ascontiguousarray(
        np.asarray(inputs["features"], dtype=np.float32).reshape(B * M, D))

    if _compiled is None:
        _compiled = (_build(), _host_consts())
    nc, consts = _compiled

    in_maps = []
    for k in range(NCORES):
        im = dict(consts)
        im["f"] = feats[k * ROWS:(k + 1) * ROWS]
        in_maps.append(im)

    trace = bool(os.environ.get("BASS_TRACE"))
    if trace:
        _ensure_axon_hooks()
    try:
        res = bass_utils.run_bass_kernel_spmd(
            nc, in_maps, core_ids=list(range(NCORES)), trace=trace)
    except Exception:
        if not trace:
            raise
        # Tracing plumbing failed; rerun untraced so the result is still valid.
        os.environ["BASS_NEVER_TRACE"] = "1"
        try:
            res = bass_utils.run_bass_kernel_spmd(
                nc, in_maps, core_ids=list(range(NCORES)), trace=False)
        finally:
            del os.environ["BASS_NEVER_TRACE"]
    LAST_RESULTS = res
    total = float(np.sum([np.float64(r["out"][0, 0]) for r in res.results]))
    return np.array(total, dtype=np.float32)
